# revision 7
# baseline (speedup 1.0000x reference)
"""CycleConsistencyLoss on 8 Trainium2 NeuronCores (Bass/Tile, SPMD data-parallel).

Math (per batch, clip [M,D], sent [N,D], prefix masks):
  soft_nn(src,tgt): w = softmax_j(-dist(src_i,tgt_j) masked); nn = w @ tgt
  dist = (|s|^2+|t|^2-2 s.t)/D; softmax shift-invariance =>
  w[i,j] prop exp((2 s_i.t_j - |t_j|^2)/D) * mask_j
  index_nn = sum_u u*beta / sum_u beta over tgt2 = src embeddings
  loss_c = mean_b sum_i (index_nn[i]-i)^2 * mask_i / len_b

Key structure (per slot, cycle0 = clip->sent->clip, cycle1 = sent cycle):
  S1 (cycle0 only): Et[j_s, i_c] = exp(2 s.c/D + bias_s_j)  (PE mm + ACT)
  The cycle1 score matrix is Et^T up to a per-column factor that cancels in
  the softmax normalization, so cycle1's Et comes from DMA XBAR transposes of
  cycle0's Et -- no second score matmul / exp pass.
  S2: nn[e,i] accumulated over tgt blocks; lhsT col 0 = ones (cycle0) or
  c_j = exp(-|t_j|^2/D)*mask (cycle1, to restore the tgt factor the transpose
  lost), so psum row 0 is the softmax denominator: no thin den matmul.
  nn rows 0..126 carry tgt dims 0..126; dim 127 is dropped from the *second*
  soft-nn only (its contribution is a soft-avg coordinate ~0.05, exponent
  error ~8e-4 -- negligible; S1 scores use all 128 dims exactly).
  C: rr = 1/den row 0; nns = nn * bcast(rr) (fp16), row 0 -> 1.
  D: dots2[u,i] = ct0_ub^T nns (ct0 row 127 = 0); Bt = exp(scale*dots2 +
  bias_u); thin [ones|iota] matmul -> den2/num psum rows; iota <= 1023 is
  exact in fp16 (no hi/lo split).
  final: index_nn = num/den2; batched loss over [8 units, 2, 512]; host avgs.

Mask penalty PEN=-12 keeps masked exp values tiny-but-nonzero in fp16 so the
transposed zero-ish columns never produce den=0 (recip_approx_fast(0)=NaN).
"""
import sys

sys.path.insert(0, "/opt/trn_rl_repo")

import numpy as np

import concourse.bass as bass
import concourse.tile as tile
from concourse import bacc, mybir
from concourse.bass_utils import run_bass_kernel_spmd

F32 = mybir.dt.float32
FP16 = mybir.dt.float16
EXP = mybir.ActivationFunctionType.Exp
ALU = mybir.AluOpType

B, M, N, D = 32, 1024, 1024, 128
NB = M // 128
NCORES = 8
SLOTS = B // NCORES  # 4
NUNITS = 2 * SLOTS
PEN = -12.0  # exp(PEN+x) ~ 1e-5: tiny but nonzero in fp16 (NaN-safe recip)

_PROGRAM_CACHE = {}
LAST_RESULT = None


def _chunks(ext):
    """512-wide chunks of the i extent."""
    if ext <= 512:
        return [(0, ext)]
    return [(0, 512), (512, ext - 512)]


def _emit(nc, tc, ctx, io, plans):
    scale = 2.0 / D

    const = ctx.enter_context(tc.tile_pool(name="const", bufs=1))
    emb = ctx.enter_context(tc.tile_pool(name="emb", bufs=2))
    etp = ctx.enter_context(tc.tile_pool(name="etp", bufs=2))
    etc = ctx.enter_context(tc.tile_pool(name="etc", bufs=2))
    nnsp = ctx.enter_context(tc.tile_pool(name="nnsp", bufs=3))
    btp = ctx.enter_context(tc.tile_pool(name="btp", bufs=4))
    rrp = ctx.enter_context(tc.tile_pool(name="rrp", bufs=2))
    bcp = ctx.enter_context(tc.tile_pool(name="bcp", bufs=2))
    fin = ctx.enter_context(tc.tile_pool(name="fin", bufs=1))

    ps_big = ctx.enter_context(tc.tile_pool(name="ps_big", bufs=2, space="PSUM"))
    ps_nn = ctx.enter_context(tc.tile_pool(name="ps_nn", bufs=3, space="PSUM"))
    ps_th = ctx.enter_context(tc.tile_pool(name="ps_th", bufs=1, space="PSUM"))

    thin2w = const.tile([128, NB, 2], FP16, tag="thin2w")
    nc.sync.dma_start(out=thin2w, in_=io["thin2w"])
    iota_t = const.tile([NUNITS, 2, 512], F32, tag="iota")
    nc.sync.dma_start(out=iota_t, in_=io["iota8"].rearrange("r (q x) -> r q x", q=2))
    masks_t = const.tile([NUNITS, 2, 512], F32, tag="masks")
    nc.sync.dma_start(out=masks_t, in_=io["masks8"].rearrange("r (q x) -> r q x", q=2))
    rlens_t = const.tile([NUNITS, 1], F32, tag="rlens")
    nc.sync.dma_start(out=rlens_t, in_=io["rlens"])

    # staging rows 0,1 = q0 den/num; rows 32,33 = q1 (matmul base-partition
    # rule); memset 1.0 covers unwritten
    th_sb = fin.tile([34, NUNITS, 512], F32, tag="th_sb")
    nc.vector.memset(th_sb, 1.0)

    slot_tiles = {}

    def get_slot(s):
        if s in slot_tiles:
            return slot_tiles[s]
        t = {}
        for name, shape, dt in [
            ("ct", [128, M], FP16), ("st", [128, N], FP16),
            ("ct0", [128, M], FP16), ("st0", [128, N], FP16),
            ("xna", [128, NB, 128], FP16), ("xnb", [128, NB, 128], FP16),
            ("bias_c", [128, NB], F32), ("bias_s", [128, NB], F32),
        ]:
            t[name] = emb.tile(shape, dt, tag=name, name=f"{name}{s}")
            nc.sync.dma_start(out=t[name], in_=io[name][s])
        slot_tiles[s] = t
        return t

    def s2_c_phase(k, nb_t, se, Etile, lhs_nn):
        """nn accumulation + normalize; returns {off: nns_tile}."""
        nntiles = {}
        for off, w in _chunks(se):
            nnp = ps_nn.tile([128, 512], F32, tag="nn", name=f"nn_{k}_{off}")
            for tb in range(nb_t):
                nc.tensor.matmul(nnp[:, 0:w], lhsT=lhs_nn[:, tb, :],
                                 rhs=Etile[:, tb, off:off + w],
                                 start=tb == 0, stop=tb == nb_t - 1)
            rr = rrp.tile([1, 512], F32, tag="rr")
            nc.vector.reciprocal_approx_fast(out=rr[:, 0:w], in_=nnp[0:1, 0:w])
            bc = bcp.tile([128, 512], F32, tag="bc")
            nc.gpsimd.partition_broadcast(bc[:, 0:w], rr[:, 0:w])
            nns = nnsp.tile([128, 512], FP16, tag="nns")
            nc.vector.scalar_tensor_tensor(nns[:, 0:w], in0=nnp[:, 0:w],
                                           scalar=1.0, in1=bc[:, 0:w],
                                           op0=ALU.bypass, op1=ALU.mult)
            nntiles[off] = nns
        return nntiles

    def d_phase(k, nb_u, se, nntiles, Dlhs, bias_d):
        """dots2 -> exp -> [ones|iota] thin matmuls -> th psum rows."""
        th = ps_th.tile([34, 512], F32, tag="th", name=f"th_{k}")
        pend = []  # software pipeline: delay thin2 by LAG ubs to hide ACT

        def flush(ub):
            for qi, (off, w) in enumerate(_chunks(se)):
                nc.tensor.matmul(th[32 * qi:32 * qi + 2, 0:w],
                                 lhsT=thin2w[:, ub, :],
                                 rhs=pend[ub][:, off:off + w],
                                 start=ub == 0, stop=ub == nb_u - 1)

        for ub in range(nb_u):
            big2 = ps_big.tile([128, 1024], F32, tag="big", name=f"d_{k}_{ub}")
            for off, w in _chunks(se):
                nc.tensor.matmul(big2[:, off:off + w],
                                 lhsT=Dlhs[:, 128 * ub:128 * (ub + 1)],
                                 rhs=nntiles[off][:, 0:w], start=True, stop=True)
            bt = btp.tile([128, 1024], FP16, tag="bt")
            nc.scalar.activation(bt[:, 0:se], big2[:, 0:se], EXP,
                                 bias=bias_d[:, ub:ub + 1], scale=scale)
            pend.append(bt)
            if ub >= 2:
                flush(ub - 2)
        for ub in range(max(0, nb_u - 2), nb_u):
            flush(ub)
        for qi, (off, w) in enumerate(_chunks(se)):
            nc.vector.tensor_copy(th_sb[32 * qi:32 * qi + 2, k, 0:w],
                                  th[32 * qi:32 * qi + 2, 0:w])

    for s in range(SLOTS):
        cb, sb = plans[s]
        se0, se1 = cb * 128, sb * 128
        t = get_slot(s)
        k0, k1 = 2 * s, 2 * s + 1

        # S1: cycle0 scores+exp; transposes feed cycle1
        et = etp.tile([128, sb, se0], FP16, tag="et", name=f"et{s}")
        ett = etc.tile([128, cb, se1], FP16, tag="ett", name=f"ett{s}")
        for tb in range(sb):
            big = ps_big.tile([128, 1024], F32, tag="big", name=f"s1_{s}_{tb}")
            for off, w in _chunks(se0):
                nc.tensor.matmul(big[:, off:off + w],
                                 lhsT=t["st"][:, 128 * tb:128 * (tb + 1)],
                                 rhs=t["ct"][:, off:off + w], start=True, stop=True)
            nc.scalar.activation(et[:, tb, 0:se0], big[:, 0:se0], EXP,
                                 bias=t["bias_s"][:, tb:tb + 1], scale=scale)
            for ub in range(cb):
                nc.sync.dma_start_transpose(
                    out=ett[:, ub, 128 * tb:128 * (tb + 1)],
                    in_=et[:, tb, 128 * ub:128 * (ub + 1)])

        # cycle0 unit
        nnt = s2_c_phase(k0, sb, se0, et, t["xna"])
        d_phase(k0, cb, se0, nnt, t["ct0"], t["bias_c"])
        # cycle1 unit (Et from transposes)
        nnt = s2_c_phase(k1, cb, se1, ett, t["xnb"])
        d_phase(k1, sb, se1, nnt, t["st0"], t["bias_s"])

    # ---- final: batched loss over [units, 2, 512] ----
    den8 = fin.tile([NUNITS, 2, 512], F32, tag="den8")
    num8 = fin.tile([NUNITS, 2, 512], F32, tag="num8")
    nc.sync.dma_start(out=den8[:, 0, :], in_=th_sb[0:1, :, :])
    nc.sync.dma_start(out=den8[:, 1, :], in_=th_sb[32:33, :, :])
    nc.sync.dma_start(out=num8[:, 0, :], in_=th_sb[1:2, :, :])
    nc.sync.dma_start(out=num8[:, 1, :], in_=th_sb[33:34, :, :])
    rden = fin.tile([NUNITS, 2, 512], F32, tag="rden")
    scr = fin.tile([NUNITS, 2, 512], F32, tag="scr")
    nc.vector.reciprocal_approx_accurate(out=rden, in_=den8, scratch=scr)
    idx = fin.tile([NUNITS, 2, 512], F32, tag="idx")
    nc.vector.tensor_mul(idx, num8, rden)
    ierr = fin.tile([NUNITS, 2, 512], F32, tag="ierr")
    nc.vector.tensor_sub(ierr, idx, iota_t)
    tmp = fin.tile([NUNITS, 2, 512], F32, tag="tmp")
    nc.vector.tensor_mul(tmp, ierr, masks_t)
    sq = fin.tile([NUNITS, 2, 512], F32, tag="sq")
    sums = fin.tile([NUNITS, 1], F32, tag="sums")
    nc.vector.scalar_tensor_tensor(sq, in0=tmp, scalar=1.0, in1=ierr,
                                   op0=ALU.bypass, op1=ALU.mult, accum_out=sums)
    loss = fin.tile([NUNITS, 1], F32, tag="loss")
    nc.vector.tensor_mul(loss, sums, rlens_t)
    nc.sync.dma_start(out=io["loss8"], in_=loss)


def _build_program(plans):
    key = tuple(plans)
    if key in _PROGRAM_CACHE:
        return _PROGRAM_CACHE[key]
    nc = bacc.Bacc("TRN2", target_bir_lowering=False, debug=False,
                   num_devices=NCORES)
    io = {
        "ct": nc.dram_tensor("ct", [SLOTS, D, M], FP16, kind="ExternalInput").ap(),
        "st": nc.dram_tensor("st", [SLOTS, D, N], FP16, kind="ExternalInput").ap(),
        "ct0": nc.dram_tensor("ct0", [SLOTS, D, M], FP16, kind="ExternalInput").ap(),
        "st0": nc.dram_tensor("st0", [SLOTS, D, N], FP16, kind="ExternalInput").ap(),
        "xna": nc.dram_tensor("xna", [SLOTS, 128, NB, 128], FP16, kind="ExternalInput").ap(),
        "xnb": nc.dram_tensor("xnb", [SLOTS, 128, NB, 128], FP16, kind="ExternalInput").ap(),
        "bias_c": nc.dram_tensor("bias_c", [SLOTS, 128, NB], F32, kind="ExternalInput").ap(),
        "bias_s": nc.dram_tensor("bias_s", [SLOTS, 128, NB], F32, kind="ExternalInput").ap(),
        "thin2w": nc.dram_tensor("thin2w", [128, NB, 2], FP16, kind="ExternalInput").ap(),
        "iota8": nc.dram_tensor("iota8", [NUNITS, M], F32, kind="ExternalInput").ap(),
        "masks8": nc.dram_tensor("masks8", [NUNITS, M], F32, kind="ExternalInput").ap(),
        "rlens": nc.dram_tensor("rlens", [NUNITS, 1], F32, kind="ExternalInput").ap(),
        "loss8": nc.dram_tensor("loss8", [NUNITS, 1], F32, kind="ExternalOutput").ap(),
    }
    from contextlib import ExitStack
    with tile.TileContext(nc) as tc:
        with ExitStack() as ctx:
            _emit(nc, tc, ctx, io, plans)
    nc.compile()
    _PROGRAM_CACHE[key] = nc
    return nc


def _host_prep(clip_emb, clip_mask, clip_lens, sent_emb, sent_mask, sent_lens):
    """Sorted batch->(core,slot) assignment, per-slot plans, per-core inputs."""
    cb_all = np.ceil(clip_lens / 128).astype(int)
    sb_all = np.ceil(sent_lens / 128).astype(int)
    order = np.argsort(-(cb_all + sb_all) * 1000 - cb_all)  # big batches first
    plans = []
    assign = {}
    for s in range(SLOTS):
        grp = order[8 * s:8 * s + 8]
        plans.append((int(cb_all[grp].max()), int(sb_all[grp].max())))
        for core, b in enumerate(grp):
            assign[(core, s)] = int(b)

    sq_c = np.einsum("bmd,bmd->bm", clip_emb, clip_emb)
    sq_s = np.einsum("bnd,bnd->bn", sent_emb, sent_emb)
    bias_c = (-sq_c / D + PEN * (1.0 - clip_mask)).astype(np.float32)
    bias_s = (-sq_s / D + PEN * (1.0 - sent_mask)).astype(np.float32)
    cw_c = (np.exp(-sq_c / D) * clip_mask).astype(np.float32)  # cycle1 tgt wts

    thin2w = np.zeros((128, NB, 2), np.float16)
    thin2w[:, :, 0] = 1.0
    thin2w[:, :, 1] = (np.arange(128)[:, None] + 128 * np.arange(NB)[None, :])
    iota8 = np.broadcast_to(np.arange(M, dtype=np.float32), (NUNITS, M)).copy()

    in_maps = []
    for core in range(NCORES):
        bs = [assign[(core, s)] for s in range(SLOTS)]
        ce = clip_emb[bs]
        se = sent_emb[bs]
        ct = np.ascontiguousarray(ce.transpose(0, 2, 1)).astype(np.float16)
        st = np.ascontiguousarray(se.transpose(0, 2, 1)).astype(np.float16)
        ct0 = np.zeros_like(ct)
        ct0[:, 1:, :] = ct[:, :127, :]
        st0 = np.zeros_like(st)
        st0[:, 1:, :] = st[:, :127, :]
        xt = se.reshape(SLOTS, NB, 128, D).transpose(0, 2, 1, 3)
        xna = np.zeros((SLOTS, 128, NB, D), np.float16)
        xna[..., 1:] = xt[..., :127]
        xna[..., 0] = 1.0
        cec = ce * cw_c[bs][..., None]
        xbt = cec.reshape(SLOTS, NB, 128, D).transpose(0, 2, 1, 3)
        xnb = np.zeros((SLOTS, 128, NB, D), np.float16)
        xnb[..., 1:] = xbt[..., :127]
        xnb[..., 0] = cw_c[bs].reshape(SLOTS, NB, 128).transpose(0, 2, 1)

        masks8 = np.empty((NUNITS, M), np.float32)
        rlens = np.empty((NUNITS, 1), np.float32)
        for s, b in enumerate(bs):
            masks8[2 * s + 0] = clip_mask[b]
            masks8[2 * s + 1] = sent_mask[b]
            rlens[2 * s + 0] = 1.0 / clip_lens[b]
            rlens[2 * s + 1] = 1.0 / sent_lens[b]
        in_maps.append({
            "ct": ct, "st": st, "ct0": ct0, "st0": st0,
            "xna": xna, "xnb": xnb,
            "bias_c": np.ascontiguousarray(
                bias_c[bs].reshape(SLOTS, NB, 128).transpose(0, 2, 1)),
            "bias_s": np.ascontiguousarray(
                bias_s[bs].reshape(SLOTS, NB, 128).transpose(0, 2, 1)),
            "thin2w": thin2w,
            "iota8": iota8,
            "masks8": masks8,
            "rlens": rlens,
        })
    return in_maps, assign, plans


def kernel(clip_emb, clip_mask, clip_lens, sent_emb, sent_mask, sent_lens):
    global LAST_RESULT
    clip_emb = np.asarray(clip_emb, np.float32)
    sent_emb = np.asarray(sent_emb, np.float32)
    clip_mask = np.asarray(clip_mask, np.float32)
    sent_mask = np.asarray(sent_mask, np.float32)
    clip_lens = np.asarray(clip_lens, np.float32)
    sent_lens = np.asarray(sent_lens, np.float32)

    in_maps, _, plans = _host_prep(clip_emb, clip_mask, clip_lens,
                                   sent_emb, sent_mask, sent_lens)
    nc = _build_program(plans)
    res = run_bass_kernel_spmd(nc, in_maps, list(range(NCORES)))
    LAST_RESULT = res

    rows = np.stack([res.results[c]["loss8"].reshape(NUNITS) for c in range(NCORES)])
    clip_loss = rows[:, 0::2].mean()
    sent_loss = rows[:, 1::2].mean()
    return (np.float32(clip_loss), np.float32(sent_loss))


# revision 8
# speedup vs baseline: 2.1612x; 2.1612x over previous
"""CycleConsistencyLoss on 8 Trainium2 NeuronCores (Bass/Tile, SPMD data-parallel).

Math (per batch, clip [M,D], sent [N,D], prefix masks):
  soft_nn(src,tgt): w = softmax_j(-dist(src_i,tgt_j) masked); nn = w @ tgt
  dist = (|s|^2+|t|^2-2 s.t)/D; softmax shift-invariance =>
  w[i,j] prop exp((2 s_i.t_j - |t_j|^2)/D) * mask_j
  index_nn = sum_u u*beta / sum_u beta over tgt2 = src embeddings
  loss_c = mean_b sum_i (index_nn[i]-i)^2 * mask_i / len_b

Key structure (per slot, cycle0 = clip->sent->clip, cycle1 = sent cycle):
  S1: Et[j_t, i] = exp(2 t.s/D + bias_t_j)  (PE mm + fused full-width ACT)
  (DMA-XBAR-transposing cycle0's Et for cycle1 was tried and lost: 234
  [128,128] transposes at ~1.2us dispatch each serialize the SP queue.)
  S2: nn[e,i] accumulated over tgt blocks; lhsT col 0 = ones, so psum row 0
  is the softmax denominator: no thin den matmul.
  nn rows 0..126 carry tgt dims 0..126; dim 127 is dropped from the *second*
  soft-nn only (its contribution is a soft-avg coordinate ~0.05, exponent
  error ~8e-4 -- negligible; S1 scores use all 128 dims exactly).
  C: rr = 1/den row 0; nns = nn * bcast(rr) (fp16), row 0 -> 1.
  D: dots2[u,i] = ct0_ub^T nns (ct0 row 127 = 0); Bt = exp(scale*dots2 +
  bias_u); thin [ones|iota] matmul -> den2/num psum rows; iota <= 1023 is
  exact in fp16 (no hi/lo split).
  final: index_nn = num/den2; batched loss over [8 units, 2, 512]; host avgs.

Mask penalty PEN=-12 keeps masked exp values tiny-but-nonzero in fp16
(recip_approx_fast(0)=NaN; den stays finite everywhere).
"""
import sys

sys.path.insert(0, "/opt/trn_rl_repo")

import numpy as np

import concourse.bass as bass
import concourse.tile as tile
from concourse import bacc, mybir
from concourse.bass_utils import run_bass_kernel_spmd

F32 = mybir.dt.float32
FP16 = mybir.dt.float16
EXP = mybir.ActivationFunctionType.Exp
ALU = mybir.AluOpType

B, M, N, D = 32, 1024, 1024, 128
NB = M // 128
NCORES = 8
SLOTS = B // NCORES  # 4
NUNITS = 2 * SLOTS
PEN = -12.0  # exp(PEN+x) ~ 1e-5: tiny but nonzero in fp16 (NaN-safe recip)

_PROGRAM_CACHE = {}
LAST_RESULT = None


def _chunks(ext):
    """512-wide chunks of the i extent."""
    if ext <= 512:
        return [(0, ext)]
    return [(0, 512), (512, ext - 512)]


def _emit(nc, tc, ctx, io, plans):
    scale = 2.0 / D

    const = ctx.enter_context(tc.tile_pool(name="const", bufs=1))
    emb = ctx.enter_context(tc.tile_pool(name="emb", bufs=2))
    etp = ctx.enter_context(tc.tile_pool(name="etp", bufs=2))
    nnsp = ctx.enter_context(tc.tile_pool(name="nnsp", bufs=3))
    btp = ctx.enter_context(tc.tile_pool(name="btp", bufs=4))
    rrp = ctx.enter_context(tc.tile_pool(name="rrp", bufs=2))
    bcp = ctx.enter_context(tc.tile_pool(name="bcp", bufs=2))
    fin = ctx.enter_context(tc.tile_pool(name="fin", bufs=1))

    ps_big = ctx.enter_context(tc.tile_pool(name="ps_big", bufs=2, space="PSUM"))
    ps_nn = ctx.enter_context(tc.tile_pool(name="ps_nn", bufs=3, space="PSUM"))
    ps_th = ctx.enter_context(tc.tile_pool(name="ps_th", bufs=1, space="PSUM"))

    thin2w = const.tile([128, NB, 2], FP16, tag="thin2w")
    nc.sync.dma_start(out=thin2w, in_=io["thin2w"])
    iota_t = const.tile([NUNITS, 2, 512], F32, tag="iota")
    nc.sync.dma_start(out=iota_t, in_=io["iota8"].rearrange("r (q x) -> r q x", q=2))
    masks_t = const.tile([NUNITS, 2, 512], F32, tag="masks")
    nc.sync.dma_start(out=masks_t, in_=io["masks8"].rearrange("r (q x) -> r q x", q=2))
    rlens_t = const.tile([NUNITS, 1], F32, tag="rlens")
    nc.sync.dma_start(out=rlens_t, in_=io["rlens"])

    # staging rows 0,1 = q0 den/num; rows 32,33 = q1 (matmul base-partition
    # rule); memset 1.0 covers unwritten
    th_sb = fin.tile([34, NUNITS, 512], F32, tag="th_sb")
    nc.vector.memset(th_sb, 1.0)

    slot_tiles = {}

    def get_slot(s):
        if s in slot_tiles:
            return slot_tiles[s]
        t = {}
        for name, shape, dt in [
            ("ct", [128, M], FP16), ("st", [128, N], FP16),
            ("ct0", [128, M], FP16), ("st0", [128, N], FP16),
            ("xna", [128, NB, 128], FP16), ("xnb", [128, NB, 128], FP16),
            ("bias_c", [128, NB], F32), ("bias_s", [128, NB], F32),
        ]:
            t[name] = emb.tile(shape, dt, tag=name, name=f"{name}{s}")
            nc.sync.dma_start(out=t[name], in_=io[name][s])
        slot_tiles[s] = t
        return t

    def s2_c_phase(k, nb_t, se, Etile, lhs_nn):
        """nn accumulation + normalize; returns {off: nns_tile}."""
        nntiles = {}
        for off, w in _chunks(se):
            nnp = ps_nn.tile([128, 512], F32, tag="nn", name=f"nn_{k}_{off}")
            for tb in range(nb_t):
                nc.tensor.matmul(nnp[:, 0:w], lhsT=lhs_nn[:, tb, :],
                                 rhs=Etile[:, tb, off:off + w],
                                 start=tb == 0, stop=tb == nb_t - 1)
            rr = rrp.tile([1, 512], F32, tag="rr")
            nc.vector.reciprocal_approx_fast(out=rr[:, 0:w], in_=nnp[0:1, 0:w])
            bc = bcp.tile([128, 512], F32, tag="bc")
            nc.gpsimd.partition_broadcast(bc[:, 0:w], rr[:, 0:w])
            nns = nnsp.tile([128, 512], FP16, tag="nns")
            nc.vector.scalar_tensor_tensor(nns[:, 0:w], in0=nnp[:, 0:w],
                                           scalar=1.0, in1=bc[:, 0:w],
                                           op0=ALU.bypass, op1=ALU.mult)
            nntiles[off] = nns
        return nntiles

    def d_phase(k, nb_u, se, nntiles, Dlhs, bias_d):
        """dots2 -> exp -> [ones|iota] thin matmuls -> th psum rows."""
        th = ps_th.tile([34, 512], F32, tag="th", name=f"th_{k}")
        pend = []  # software pipeline: delay thin2 by LAG ubs to hide ACT

        def flush(ub):
            for qi, (off, w) in enumerate(_chunks(se)):
                nc.tensor.matmul(th[32 * qi:32 * qi + 2, 0:w],
                                 lhsT=thin2w[:, ub, :],
                                 rhs=pend[ub][:, off:off + w],
                                 start=ub == 0, stop=ub == nb_u - 1)

        for ub in range(nb_u):
            big2 = ps_big.tile([128, 1024], F32, tag="big", name=f"d_{k}_{ub}")
            for off, w in _chunks(se):
                nc.tensor.matmul(big2[:, off:off + w],
                                 lhsT=Dlhs[:, 128 * ub:128 * (ub + 1)],
                                 rhs=nntiles[off][:, 0:w], start=True, stop=True)
            bt = btp.tile([128, 1024], FP16, tag="bt")
            nc.scalar.activation(bt[:, 0:se], big2[:, 0:se], EXP,
                                 bias=bias_d[:, ub:ub + 1], scale=scale)
            pend.append(bt)
            if ub >= 2:
                flush(ub - 2)
        for ub in range(max(0, nb_u - 2), nb_u):
            flush(ub)
        for qi, (off, w) in enumerate(_chunks(se)):
            nc.vector.tensor_copy(th_sb[32 * qi:32 * qi + 2, k, 0:w],
                                  th[32 * qi:32 * qi + 2, 0:w])

    def s1_phase(s, k, nb_t, se, X, Y, bias_t):
        et = etp.tile([128, NB, 1024], FP16, tag="et", name=f"et{k}")
        for tb in range(nb_t):
            big = ps_big.tile([128, 1024], F32, tag="big", name=f"s1_{k}_{tb}")
            for off, w in _chunks(se):
                nc.tensor.matmul(big[:, off:off + w],
                                 lhsT=X[:, 128 * tb:128 * (tb + 1)],
                                 rhs=Y[:, off:off + w], start=True, stop=True)
            nc.scalar.activation(et[:, tb, 0:se], big[:, 0:se], EXP,
                                 bias=bias_t[:, tb:tb + 1], scale=scale)
        return et

    for s in range(SLOTS):
        cb, sb = plans[s]
        se0, se1 = cb * 128, sb * 128
        t = get_slot(s)
        k0, k1 = 2 * s, 2 * s + 1

        et = s1_phase(s, k0, sb, se0, t["st"], t["ct"], t["bias_s"])
        nnt = s2_c_phase(k0, sb, se0, et, t["xna"])
        d_phase(k0, cb, se0, nnt, t["ct0"], t["bias_c"])

        et = s1_phase(s, k1, cb, se1, t["ct"], t["st"], t["bias_c"])
        nnt = s2_c_phase(k1, cb, se1, et, t["xnb"])
        d_phase(k1, sb, se1, nnt, t["st0"], t["bias_s"])

    # ---- final: batched loss over [units, 2, 512] ----
    den8 = fin.tile([NUNITS, 2, 512], F32, tag="den8")
    num8 = fin.tile([NUNITS, 2, 512], F32, tag="num8")
    nc.sync.dma_start(out=den8[:, 0, :], in_=th_sb[0:1, :, :])
    nc.sync.dma_start(out=den8[:, 1, :], in_=th_sb[32:33, :, :])
    nc.sync.dma_start(out=num8[:, 0, :], in_=th_sb[1:2, :, :])
    nc.sync.dma_start(out=num8[:, 1, :], in_=th_sb[33:34, :, :])
    rden = fin.tile([NUNITS, 2, 512], F32, tag="rden")
    scr = fin.tile([NUNITS, 2, 512], F32, tag="scr")
    nc.vector.reciprocal_approx_accurate(out=rden, in_=den8, scratch=scr)
    idx = fin.tile([NUNITS, 2, 512], F32, tag="idx")
    nc.vector.tensor_mul(idx, num8, rden)
    ierr = fin.tile([NUNITS, 2, 512], F32, tag="ierr")
    nc.vector.tensor_sub(ierr, idx, iota_t)
    tmp = fin.tile([NUNITS, 2, 512], F32, tag="tmp")
    nc.vector.tensor_mul(tmp, ierr, masks_t)
    sq = fin.tile([NUNITS, 2, 512], F32, tag="sq")
    sums = fin.tile([NUNITS, 1], F32, tag="sums")
    nc.vector.scalar_tensor_tensor(sq, in0=tmp, scalar=1.0, in1=ierr,
                                   op0=ALU.bypass, op1=ALU.mult, accum_out=sums)
    loss = fin.tile([NUNITS, 1], F32, tag="loss")
    nc.vector.tensor_mul(loss, sums, rlens_t)
    nc.sync.dma_start(out=io["loss8"], in_=loss)


def _build_program(plans):
    key = tuple(plans)
    if key in _PROGRAM_CACHE:
        return _PROGRAM_CACHE[key]
    nc = bacc.Bacc("TRN2", target_bir_lowering=False, debug=False,
                   num_devices=NCORES)
    io = {
        "ct": nc.dram_tensor("ct", [SLOTS, D, M], FP16, kind="ExternalInput").ap(),
        "st": nc.dram_tensor("st", [SLOTS, D, N], FP16, kind="ExternalInput").ap(),
        "ct0": nc.dram_tensor("ct0", [SLOTS, D, M], FP16, kind="ExternalInput").ap(),
        "st0": nc.dram_tensor("st0", [SLOTS, D, N], FP16, kind="ExternalInput").ap(),
        "xna": nc.dram_tensor("xna", [SLOTS, 128, NB, 128], FP16, kind="ExternalInput").ap(),
        "xnb": nc.dram_tensor("xnb", [SLOTS, 128, NB, 128], FP16, kind="ExternalInput").ap(),
        "bias_c": nc.dram_tensor("bias_c", [SLOTS, 128, NB], F32, kind="ExternalInput").ap(),
        "bias_s": nc.dram_tensor("bias_s", [SLOTS, 128, NB], F32, kind="ExternalInput").ap(),
        "thin2w": nc.dram_tensor("thin2w", [128, NB, 2], FP16, kind="ExternalInput").ap(),
        "iota8": nc.dram_tensor("iota8", [NUNITS, M], F32, kind="ExternalInput").ap(),
        "masks8": nc.dram_tensor("masks8", [NUNITS, M], F32, kind="ExternalInput").ap(),
        "rlens": nc.dram_tensor("rlens", [NUNITS, 1], F32, kind="ExternalInput").ap(),
        "loss8": nc.dram_tensor("loss8", [NUNITS, 1], F32, kind="ExternalOutput").ap(),
    }
    from contextlib import ExitStack
    with tile.TileContext(nc) as tc:
        with ExitStack() as ctx:
            _emit(nc, tc, ctx, io, plans)
    nc.compile()
    _PROGRAM_CACHE[key] = nc
    return nc


def _host_prep(clip_emb, clip_mask, clip_lens, sent_emb, sent_mask, sent_lens):
    """Sorted batch->(core,slot) assignment, per-slot plans, per-core inputs."""
    cb_all = np.ceil(clip_lens / 128).astype(int)
    sb_all = np.ceil(sent_lens / 128).astype(int)
    order = np.argsort(-(cb_all + sb_all) * 1000 - cb_all)  # big batches first
    plans = []
    assign = {}
    for s in range(SLOTS):
        grp = order[8 * s:8 * s + 8]
        plans.append((int(cb_all[grp].max()), int(sb_all[grp].max())))
        for core, b in enumerate(grp):
            assign[(core, s)] = int(b)

    sq_c = np.einsum("bmd,bmd->bm", clip_emb, clip_emb)
    sq_s = np.einsum("bnd,bnd->bn", sent_emb, sent_emb)
    bias_c = (-sq_c / D + PEN * (1.0 - clip_mask)).astype(np.float32)
    bias_s = (-sq_s / D + PEN * (1.0 - sent_mask)).astype(np.float32)
    thin2w = np.zeros((128, NB, 2), np.float16)
    thin2w[:, :, 0] = 1.0
    thin2w[:, :, 1] = (np.arange(128)[:, None] + 128 * np.arange(NB)[None, :])
    iota8 = np.broadcast_to(np.arange(M, dtype=np.float32), (NUNITS, M)).copy()

    in_maps = []
    for core in range(NCORES):
        bs = [assign[(core, s)] for s in range(SLOTS)]
        ce = clip_emb[bs]
        se = sent_emb[bs]
        ct = np.ascontiguousarray(ce.transpose(0, 2, 1)).astype(np.float16)
        st = np.ascontiguousarray(se.transpose(0, 2, 1)).astype(np.float16)
        ct0 = np.zeros_like(ct)
        ct0[:, 1:, :] = ct[:, :127, :]
        st0 = np.zeros_like(st)
        st0[:, 1:, :] = st[:, :127, :]
        xt = se.reshape(SLOTS, NB, 128, D).transpose(0, 2, 1, 3)
        xna = np.zeros((SLOTS, 128, NB, D), np.float16)
        xna[..., 1:] = xt[..., :127]
        xna[..., 0] = 1.0
        xbt = ce.reshape(SLOTS, NB, 128, D).transpose(0, 2, 1, 3)
        xnb = np.zeros((SLOTS, 128, NB, D), np.float16)
        xnb[..., 1:] = xbt[..., :127]
        xnb[..., 0] = 1.0

        masks8 = np.empty((NUNITS, M), np.float32)
        rlens = np.empty((NUNITS, 1), np.float32)
        for s, b in enumerate(bs):
            masks8[2 * s + 0] = clip_mask[b]
            masks8[2 * s + 1] = sent_mask[b]
            rlens[2 * s + 0] = 1.0 / clip_lens[b]
            rlens[2 * s + 1] = 1.0 / sent_lens[b]
        in_maps.append({
            "ct": ct, "st": st, "ct0": ct0, "st0": st0,
            "xna": xna, "xnb": xnb,
            "bias_c": np.ascontiguousarray(
                bias_c[bs].reshape(SLOTS, NB, 128).transpose(0, 2, 1)),
            "bias_s": np.ascontiguousarray(
                bias_s[bs].reshape(SLOTS, NB, 128).transpose(0, 2, 1)),
            "thin2w": thin2w,
            "iota8": iota8,
            "masks8": masks8,
            "rlens": rlens,
        })
    return in_maps, assign, plans


def kernel(clip_emb, clip_mask, clip_lens, sent_emb, sent_mask, sent_lens):
    global LAST_RESULT
    clip_emb = np.asarray(clip_emb, np.float32)
    sent_emb = np.asarray(sent_emb, np.float32)
    clip_mask = np.asarray(clip_mask, np.float32)
    sent_mask = np.asarray(sent_mask, np.float32)
    clip_lens = np.asarray(clip_lens, np.float32)
    sent_lens = np.asarray(sent_lens, np.float32)

    in_maps, _, plans = _host_prep(clip_emb, clip_mask, clip_lens,
                                   sent_emb, sent_mask, sent_lens)
    nc = _build_program(plans)
    res = run_bass_kernel_spmd(nc, in_maps, list(range(NCORES)))
    LAST_RESULT = res

    rows = np.stack([res.results[c]["loss8"].reshape(NUNITS) for c in range(NCORES)])
    clip_loss = rows[:, 0::2].mean()
    sent_loss = rows[:, 1::2].mean()
    return (np.float32(clip_loss), np.float32(sent_loss))


# revision 10
# speedup vs baseline: 2.5025x; 1.1579x over previous
"""CycleConsistencyLoss on 8 Trainium2 NeuronCores (Bass/Tile, SPMD data-parallel).

Math (per batch, clip [M,D], sent [N,D], prefix masks):
  soft_nn(src,tgt): w = softmax_j(-dist(src_i,tgt_j) masked); nn = w @ tgt
  dist = (|s|^2+|t|^2-2 s.t)/D; softmax shift-invariance =>
  w[i,j] prop exp((2 s_i.t_j - |t_j|^2)/D) * mask_j
  index_nn = sum_u u*beta / sum_u beta over tgt2 = src embeddings
  loss_c = mean_b sum_i (index_nn[i]-i)^2 * mask_i / len_b

Key structure (per slot, cycle0 = clip->sent->clip, cycle1 = sent cycle):
  S1: Et[j_t, i] = exp(2 t.s/D + bias_t_j)  (PE mm + fused full-width ACT)
  (DMA-XBAR-transposing cycle0's Et for cycle1 was tried and lost: 234
  [128,128] transposes at ~1.2us dispatch each serialize the SP queue.)
  S2: nn[e,i] accumulated over tgt blocks; lhsT col 0 = ones, so psum row 0
  is the softmax denominator: no thin den matmul.
  nn rows 0..126 carry tgt dims 0..126; dim 127 is dropped from the *second*
  soft-nn only (its contribution is a soft-avg coordinate ~0.05, exponent
  error ~8e-4 -- negligible; S1 scores use all 128 dims exactly).
  C: rr = 1/den row 0; nns = nn * bcast(rr) (fp16), row 0 -> 1.
  D: dots2[u,i] = ct0_ub^T nns (ct0 row 127 = 0); Bt = exp(scale*dots2 +
  bias_u); thin [ones|iota] matmul -> den2/num psum rows; iota <= 1023 is
  exact in fp16 (no hi/lo split).
  final: index_nn = num/den2; batched loss over [8 units, 2, 512]; host avgs.

Mask penalty PEN=-12 keeps masked exp values tiny-but-nonzero in fp16
(recip_approx_fast(0)=NaN; den stays finite everywhere).
"""
import sys

sys.path.insert(0, "/opt/trn_rl_repo")

import numpy as np

import concourse.bass as bass
import concourse.tile as tile
from concourse import bacc, mybir
from concourse.bass_utils import run_bass_kernel_spmd

F32 = mybir.dt.float32
FP16 = mybir.dt.float16
EXP = mybir.ActivationFunctionType.Exp
ALU = mybir.AluOpType

B, M, N, D = 32, 1024, 1024, 128
NB = M // 128
NCORES = 8
SLOTS = B // NCORES  # 4
NUNITS = 2 * SLOTS
PEN = -12.0  # exp(PEN+x) ~ 1e-5: tiny but nonzero in fp16 (NaN-safe recip)

_PROGRAM_CACHE = {}
LAST_RESULT = None


def _chunks(ext):
    """512-wide chunks of the i extent."""
    if ext <= 512:
        return [(0, ext)]
    return [(0, 512), (512, ext - 512)]


def _emit(nc, tc, ctx, io, plans):
    scale = 2.0 / D

    const = ctx.enter_context(tc.tile_pool(name="const", bufs=1))
    emb = ctx.enter_context(tc.tile_pool(name="emb", bufs=2))
    etp = ctx.enter_context(tc.tile_pool(name="etp", bufs=2))
    nnsp = ctx.enter_context(tc.tile_pool(name="nnsp", bufs=2))
    btp = ctx.enter_context(tc.tile_pool(name="btp", bufs=4))
    rrp = ctx.enter_context(tc.tile_pool(name="rrp", bufs=2))
    bcp = ctx.enter_context(tc.tile_pool(name="bcp", bufs=2))
    fin = ctx.enter_context(tc.tile_pool(name="fin", bufs=1))

    ps_big = ctx.enter_context(tc.tile_pool(name="ps_big", bufs=2, space="PSUM"))
    ps_nn = ctx.enter_context(tc.tile_pool(name="ps_nn", bufs=3, space="PSUM"))
    ps_th = ctx.enter_context(tc.tile_pool(name="ps_th", bufs=1, space="PSUM"))

    thin2w = const.tile([128, NB, 2], FP16, tag="thin2w")
    nc.sync.dma_start(out=thin2w, in_=io["thin2w"])
    iota_t = const.tile([NUNITS, 2, 512], F32, tag="iota")
    nc.sync.dma_start(out=iota_t, in_=io["iota8"].rearrange("r (q x) -> r q x", q=2))
    masks_t = const.tile([NUNITS, 2, 512], F32, tag="masks")
    nc.sync.dma_start(out=masks_t, in_=io["masks8"].rearrange("r (q x) -> r q x", q=2))
    rlens_t = const.tile([NUNITS, 1], F32, tag="rlens")
    nc.sync.dma_start(out=rlens_t, in_=io["rlens"])

    # staging rows 0,1 = q0 den/num; rows 32,33 = q1 (matmul base-partition
    # rule); memset 1.0 covers unwritten
    th_sb = fin.tile([34, NUNITS, 512], F32, tag="th_sb")
    nc.vector.memset(th_sb, 1.0)

    slot_tiles = {}

    def get_slot(s):
        if s in slot_tiles:
            return slot_tiles[s]
        t = {}
        for name, shape, dt in [
            ("ct", [128, M], FP16), ("st", [128, N], FP16),
            ("ct0", [128, M], FP16), ("st0", [128, N], FP16),
            ("xna", [128, NB, 128], FP16), ("xnb", [128, NB, 128], FP16),
            ("bias_c", [128, NB], F32), ("bias_s", [128, NB], F32),
        ]:
            t[name] = emb.tile(shape, dt, tag=name, name=f"{name}{s}")
            nc.sync.dma_start(out=t[name], in_=io[name][s])
        slot_tiles[s] = t
        return t

    def s2_c_phase(k, nb_t, se, Etile, lhs_nn):
        """nn accumulation (den = row 0) + normalize into one nns tile."""
        nns = nnsp.tile([128, 1024], FP16, tag="nns", name=f"nns_{k}")
        for off, w in _chunks(se):
            nnp = ps_nn.tile([128, 512], F32, tag="nn", name=f"nn_{k}_{off}")
            for tb in range(nb_t):
                nc.tensor.matmul(nnp[:, 0:w], lhsT=lhs_nn[:, tb, :],
                                 rhs=Etile[:, tb, off:off + w],
                                 start=tb == 0, stop=tb == nb_t - 1)
            rr = rrp.tile([1, 512], F32, tag="rr")
            nc.vector.reciprocal_approx_fast(out=rr[:, 0:w], in_=nnp[0:1, 0:w])
            bc = bcp.tile([128, 512], F32, tag="bc")
            nc.gpsimd.partition_broadcast(bc[:, 0:w], rr[:, 0:w])
            nc.vector.scalar_tensor_tensor(nns[:, off:off + w], in0=nnp[:, 0:w],
                                           scalar=1.0, in1=bc[:, 0:w],
                                           op0=ALU.bypass, op1=ALU.mult)
        return nns

    def s1_step(u, tb):
        big = ps_big.tile([128, 1024], F32, tag="big", name=f"s1_{u['k']}_{tb}")
        se = u["se"]
        for off, w in _chunks(se):
            nc.tensor.matmul(big[:, off:off + w],
                             lhsT=u["X"][:, 128 * tb:128 * (tb + 1)],
                             rhs=u["Y"][:, off:off + w], start=True, stop=True)
        nc.scalar.activation(u["et"][:, tb, 0:se], big[:, 0:se], EXP,
                             bias=u["bias_t"][:, tb:tb + 1], scale=scale)

    def d_step(u, ub):
        se = u["se2"]
        big2 = ps_big.tile([128, 1024], F32, tag="big", name=f"d_{u['k']}_{ub}")
        for off, w in _chunks(se):
            nc.tensor.matmul(big2[:, off:off + w],
                             lhsT=u["Dlhs"][:, 128 * ub:128 * (ub + 1)],
                             rhs=u["nns"][:, off:off + w], start=True, stop=True)
        bt = btp.tile([128, 1024], FP16, tag="bt")
        nc.scalar.activation(bt[:, 0:se], big2[:, 0:se], EXP,
                             bias=u["bias_d"][:, ub:ub + 1], scale=scale)
        u["pend"].append(bt)

    def d_flush(u, ub):
        for qi, (off, w) in enumerate(_chunks(u["se2"])):
            nc.tensor.matmul(u["th"][32 * qi:32 * qi + 2, 0:w],
                             lhsT=thin2w[:, ub, :],
                             rhs=u["pend"][ub][:, off:off + w],
                             start=ub == 0, stop=ub == u["nb_u"] - 1)

    units = []
    for s in range(SLOTS):
        cb, sb = plans[s]
        se0, se1 = cb * 128, sb * 128
        t = get_slot(s)
        units.append(dict(k=2 * s, nb_t=sb, nb_u=cb, se=se0, se2=se0,
                          X=t["st"], Y=t["ct"], bias_t=t["bias_s"],
                          xn=t["xna"], Dlhs=t["ct0"], bias_d=t["bias_c"]))
        units.append(dict(k=2 * s + 1, nb_t=cb, nb_u=sb, se=se1, se2=se1,
                          X=t["ct"], Y=t["st"], bias_t=t["bias_c"],
                          xn=t["xnb"], Dlhs=t["st0"], bias_d=t["bias_s"]))

    # prologue: S1 of unit 0
    units[0]["et"] = etp.tile([128, NB, 1024], FP16, tag="et", name="et0")
    for tb in range(units[0]["nb_t"]):
        s1_step(units[0], tb)

    LAG = 2
    for j, u in enumerate(units):
        u["nns"] = s2_c_phase(u["k"], u["nb_t"], u["se"], u["et"], u["xn"])
        u["th"] = ps_th.tile([34, 512], F32, tag="th", name=f"th_{u['k']}")
        u["pend"] = []
        nxt = units[j + 1] if j + 1 < len(units) else None
        if nxt is not None:
            nxt["et"] = etp.tile([128, NB, 1024], FP16, tag="et",
                                 name=f"et{nxt['k']}")
        # merged loop: S1 of next unit fills PE while D of current unit
        # ping-pongs with ACT; thin2 lags its act by LAG ubs
        for i in range(max(nxt["nb_t"] if nxt else 0, u["nb_u"])):
            if nxt is not None and i < nxt["nb_t"]:
                s1_step(nxt, i)
            if i < u["nb_u"]:
                d_step(u, i)
                if i >= LAG:
                    d_flush(u, i - LAG)
        for ub in range(max(0, u["nb_u"] - LAG), u["nb_u"]):
            d_flush(u, ub)
        for qi, (off, w) in enumerate(_chunks(u["se2"])):
            nc.vector.tensor_copy(th_sb[32 * qi:32 * qi + 2, u["k"], 0:w],
                                  u["th"][32 * qi:32 * qi + 2, 0:w])

    # ---- final: batched loss over [units, 2, 512] ----
    den8 = fin.tile([NUNITS, 2, 512], F32, tag="den8")
    num8 = fin.tile([NUNITS, 2, 512], F32, tag="num8")
    nc.sync.dma_start(out=den8[:, 0, :], in_=th_sb[0:1, :, :])
    nc.sync.dma_start(out=den8[:, 1, :], in_=th_sb[32:33, :, :])
    nc.sync.dma_start(out=num8[:, 0, :], in_=th_sb[1:2, :, :])
    nc.sync.dma_start(out=num8[:, 1, :], in_=th_sb[33:34, :, :])
    rden = fin.tile([NUNITS, 2, 512], F32, tag="rden")
    scr = fin.tile([NUNITS, 2, 512], F32, tag="scr")
    nc.vector.reciprocal_approx_accurate(out=rden, in_=den8, scratch=scr)
    idx = fin.tile([NUNITS, 2, 512], F32, tag="idx")
    nc.vector.tensor_mul(idx, num8, rden)
    ierr = fin.tile([NUNITS, 2, 512], F32, tag="ierr")
    nc.vector.tensor_sub(ierr, idx, iota_t)
    tmp = fin.tile([NUNITS, 2, 512], F32, tag="tmp")
    nc.vector.tensor_mul(tmp, ierr, masks_t)
    sq = fin.tile([NUNITS, 2, 512], F32, tag="sq")
    sums = fin.tile([NUNITS, 1], F32, tag="sums")
    nc.vector.scalar_tensor_tensor(sq, in0=tmp, scalar=1.0, in1=ierr,
                                   op0=ALU.bypass, op1=ALU.mult, accum_out=sums)
    loss = fin.tile([NUNITS, 1], F32, tag="loss")
    nc.vector.tensor_mul(loss, sums, rlens_t)
    nc.sync.dma_start(out=io["loss8"], in_=loss)


def _build_program(plans):
    key = tuple(plans)
    if key in _PROGRAM_CACHE:
        return _PROGRAM_CACHE[key]
    nc = bacc.Bacc("TRN2", target_bir_lowering=False, debug=False,
                   num_devices=NCORES)
    io = {
        "ct": nc.dram_tensor("ct", [SLOTS, D, M], FP16, kind="ExternalInput").ap(),
        "st": nc.dram_tensor("st", [SLOTS, D, N], FP16, kind="ExternalInput").ap(),
        "ct0": nc.dram_tensor("ct0", [SLOTS, D, M], FP16, kind="ExternalInput").ap(),
        "st0": nc.dram_tensor("st0", [SLOTS, D, N], FP16, kind="ExternalInput").ap(),
        "xna": nc.dram_tensor("xna", [SLOTS, 128, NB, 128], FP16, kind="ExternalInput").ap(),
        "xnb": nc.dram_tensor("xnb", [SLOTS, 128, NB, 128], FP16, kind="ExternalInput").ap(),
        "bias_c": nc.dram_tensor("bias_c", [SLOTS, 128, NB], F32, kind="ExternalInput").ap(),
        "bias_s": nc.dram_tensor("bias_s", [SLOTS, 128, NB], F32, kind="ExternalInput").ap(),
        "thin2w": nc.dram_tensor("thin2w", [128, NB, 2], FP16, kind="ExternalInput").ap(),
        "iota8": nc.dram_tensor("iota8", [NUNITS, M], F32, kind="ExternalInput").ap(),
        "masks8": nc.dram_tensor("masks8", [NUNITS, M], F32, kind="ExternalInput").ap(),
        "rlens": nc.dram_tensor("rlens", [NUNITS, 1], F32, kind="ExternalInput").ap(),
        "loss8": nc.dram_tensor("loss8", [NUNITS, 1], F32, kind="ExternalOutput").ap(),
    }
    from contextlib import ExitStack
    with tile.TileContext(nc) as tc:
        with ExitStack() as ctx:
            _emit(nc, tc, ctx, io, plans)
    nc.compile()
    _PROGRAM_CACHE[key] = nc
    return nc


def _host_prep(clip_emb, clip_mask, clip_lens, sent_emb, sent_mask, sent_lens):
    """Sorted batch->(core,slot) assignment, per-slot plans, per-core inputs."""
    cb_all = np.ceil(clip_lens / 128).astype(int)
    sb_all = np.ceil(sent_lens / 128).astype(int)
    order = np.argsort(-(cb_all + sb_all) * 1000 - cb_all)  # big batches first
    plans = []
    assign = {}
    for s in range(SLOTS):
        grp = order[8 * s:8 * s + 8]
        plans.append((int(cb_all[grp].max()), int(sb_all[grp].max())))
        for core, b in enumerate(grp):
            assign[(core, s)] = int(b)

    sq_c = np.einsum("bmd,bmd->bm", clip_emb, clip_emb)
    sq_s = np.einsum("bnd,bnd->bn", sent_emb, sent_emb)
    bias_c = (-sq_c / D + PEN * (1.0 - clip_mask)).astype(np.float32)
    bias_s = (-sq_s / D + PEN * (1.0 - sent_mask)).astype(np.float32)
    thin2w = np.zeros((128, NB, 2), np.float16)
    thin2w[:, :, 0] = 1.0
    thin2w[:, :, 1] = (np.arange(128)[:, None] + 128 * np.arange(NB)[None, :])
    iota8 = np.broadcast_to(np.arange(M, dtype=np.float32), (NUNITS, M)).copy()

    in_maps = []
    for core in range(NCORES):
        bs = [assign[(core, s)] for s in range(SLOTS)]
        ce = clip_emb[bs]
        se = sent_emb[bs]
        ct = np.ascontiguousarray(ce.transpose(0, 2, 1)).astype(np.float16)
        st = np.ascontiguousarray(se.transpose(0, 2, 1)).astype(np.float16)
        ct0 = np.zeros_like(ct)
        ct0[:, 1:, :] = ct[:, :127, :]
        st0 = np.zeros_like(st)
        st0[:, 1:, :] = st[:, :127, :]
        xt = se.reshape(SLOTS, NB, 128, D).transpose(0, 2, 1, 3)
        xna = np.zeros((SLOTS, 128, NB, D), np.float16)
        xna[..., 1:] = xt[..., :127]
        xna[..., 0] = 1.0
        xbt = ce.reshape(SLOTS, NB, 128, D).transpose(0, 2, 1, 3)
        xnb = np.zeros((SLOTS, 128, NB, D), np.float16)
        xnb[..., 1:] = xbt[..., :127]
        xnb[..., 0] = 1.0

        masks8 = np.empty((NUNITS, M), np.float32)
        rlens = np.empty((NUNITS, 1), np.float32)
        for s, b in enumerate(bs):
            masks8[2 * s + 0] = clip_mask[b]
            masks8[2 * s + 1] = sent_mask[b]
            rlens[2 * s + 0] = 1.0 / clip_lens[b]
            rlens[2 * s + 1] = 1.0 / sent_lens[b]
        in_maps.append({
            "ct": ct, "st": st, "ct0": ct0, "st0": st0,
            "xna": xna, "xnb": xnb,
            "bias_c": np.ascontiguousarray(
                bias_c[bs].reshape(SLOTS, NB, 128).transpose(0, 2, 1)),
            "bias_s": np.ascontiguousarray(
                bias_s[bs].reshape(SLOTS, NB, 128).transpose(0, 2, 1)),
            "thin2w": thin2w,
            "iota8": iota8,
            "masks8": masks8,
            "rlens": rlens,
        })
    return in_maps, assign, plans


def kernel(clip_emb, clip_mask, clip_lens, sent_emb, sent_mask, sent_lens):
    global LAST_RESULT
    clip_emb = np.asarray(clip_emb, np.float32)
    sent_emb = np.asarray(sent_emb, np.float32)
    clip_mask = np.asarray(clip_mask, np.float32)
    sent_mask = np.asarray(sent_mask, np.float32)
    clip_lens = np.asarray(clip_lens, np.float32)
    sent_lens = np.asarray(sent_lens, np.float32)

    in_maps, _, plans = _host_prep(clip_emb, clip_mask, clip_lens,
                                   sent_emb, sent_mask, sent_lens)
    nc = _build_program(plans)
    res = run_bass_kernel_spmd(nc, in_maps, list(range(NCORES)))
    LAST_RESULT = res

    rows = np.stack([res.results[c]["loss8"].reshape(NUNITS) for c in range(NCORES)])
    clip_loss = rows[:, 0::2].mean()
    sent_loss = rows[:, 1::2].mean()
    return (np.float32(clip_loss), np.float32(sent_loss))


# revision 11
# speedup vs baseline: 2.7992x; 1.1186x over previous
"""CycleConsistencyLoss on 8 Trainium2 NeuronCores (Bass/Tile, SPMD data-parallel).

Math (per batch, clip [M,D], sent [N,D], prefix masks):
  soft_nn(src,tgt): w = softmax_j(-dist(src_i,tgt_j) masked); nn = w @ tgt
  dist = (|s|^2+|t|^2-2 s.t)/D; softmax shift-invariance =>
  w[i,j] prop exp((2 s_i.t_j - |t_j|^2)/D) * mask_j
  index_nn = sum_u u*beta / sum_u beta over tgt2 = src embeddings
  loss_c = mean_b sum_i (index_nn[i]-i)^2 * mask_i / len_b

Key structure (per slot, cycle0 = clip->sent->clip, cycle1 = sent cycle):
  S1: Et[j_t, i] = exp(2 t.s/D + bias_t_j)  (PE mm + fused full-width ACT)
  (DMA-XBAR-transposing cycle0's Et for cycle1 was tried and lost: 234
  [128,128] transposes at ~1.2us dispatch each serialize the SP queue.)
  S2: nn[e,i] accumulated over tgt blocks; lhsT col 0 = ones, so psum row 0
  is the softmax denominator: no thin den matmul.
  nn rows 0..126 carry tgt dims 0..126; dim 127 is dropped from the *second*
  soft-nn only (its contribution is a soft-avg coordinate ~0.05, exponent
  error ~8e-4 -- negligible; S1 scores use all 128 dims exactly).
  C: rr = 1/den row 0; nns = nn * bcast(rr) (fp16), row 0 -> 1.
  D: dots2[u,i] = ct0_ub^T nns (ct0 row 127 = 0); Bt = exp(scale*dots2 +
  bias_u); thin [ones|iota] matmul -> den2/num psum rows; iota <= 1023 is
  exact in fp16 (no hi/lo split).
  final: index_nn = num/den2; batched loss over [8 units, 2, 512]; host avgs.

Mask penalty PEN=-12 keeps masked exp values tiny-but-nonzero in fp16
(recip_approx_fast(0)=NaN; den stays finite everywhere).
"""
import sys

sys.path.insert(0, "/opt/trn_rl_repo")

import numpy as np

import concourse.bass as bass
import concourse.tile as tile
from concourse import bacc, mybir
from concourse.bass_utils import run_bass_kernel_spmd

F32 = mybir.dt.float32
FP16 = mybir.dt.float16
EXP = mybir.ActivationFunctionType.Exp
ALU = mybir.AluOpType

B, M, N, D = 32, 1024, 1024, 128
NB = M // 128
NCORES = 8
SLOTS = B // NCORES  # 4
NUNITS = 2 * SLOTS
PEN = -12.0  # exp(PEN+x) ~ 1e-5: tiny but nonzero in fp16 (NaN-safe recip)

_PROGRAM_CACHE = {}
LAST_RESULT = None


def _chunks(ext):
    """512-wide chunks of the i extent."""
    if ext <= 512:
        return [(0, ext)]
    return [(0, 512), (512, ext - 512)]


def _emit(nc, tc, ctx, io, plans):
    scale = 2.0 / D

    const = ctx.enter_context(tc.tile_pool(name="const", bufs=1))
    emb = ctx.enter_context(tc.tile_pool(name="emb", bufs=2))
    etp = ctx.enter_context(tc.tile_pool(name="etp", bufs=2))
    nnsp = ctx.enter_context(tc.tile_pool(name="nnsp", bufs=2))
    btp = ctx.enter_context(tc.tile_pool(name="btp", bufs=4))
    rrp = ctx.enter_context(tc.tile_pool(name="rrp", bufs=2))
    bcp = ctx.enter_context(tc.tile_pool(name="bcp", bufs=2))
    fin = ctx.enter_context(tc.tile_pool(name="fin", bufs=1))

    ps_big = ctx.enter_context(tc.tile_pool(name="ps_big", bufs=2, space="PSUM"))
    ps_nn = ctx.enter_context(tc.tile_pool(name="ps_nn", bufs=3, space="PSUM"))
    ps_th = ctx.enter_context(tc.tile_pool(name="ps_th", bufs=1, space="PSUM"))

    thin2w = const.tile([128, NB, 2], FP16, tag="thin2w")
    nc.sync.dma_start(out=thin2w, in_=io["thin2w"])
    iota_t = const.tile([NUNITS, 2, 512], F32, tag="iota")
    nc.sync.dma_start(out=iota_t, in_=io["iota8"].rearrange("r (q x) -> r q x", q=2))
    masks_t = const.tile([NUNITS, 2, 512], F32, tag="masks")
    nc.sync.dma_start(out=masks_t, in_=io["masks8"].rearrange("r (q x) -> r q x", q=2))
    rlens_t = const.tile([NUNITS, 1], F32, tag="rlens")
    nc.sync.dma_start(out=rlens_t, in_=io["rlens"])

    # staging rows 0,1 = q0 den/num; rows 32,33 = q1 (matmul base-partition
    # rule); memset 1.0 covers unwritten
    th_sb = fin.tile([34, NUNITS, 512], F32, tag="th_sb")
    nc.vector.memset(th_sb, 1.0)

    slot_tiles = {}

    def get_slot(s):
        if s in slot_tiles:
            return slot_tiles[s]
        t = {}
        for name, shape, dt in [
            ("ct", [128, M], FP16), ("st", [128, N], FP16),
            ("ct0", [128, M], FP16), ("st0", [128, N], FP16),
            ("xna", [128, NB, 128], FP16), ("xnb", [128, NB, 128], FP16),
            ("bias_c", [128, NB], F32), ("bias_s", [128, NB], F32),
        ]:
            t[name] = emb.tile(shape, dt, tag=name, name=f"{name}{s}")
            nc.sync.dma_start(out=t[name], in_=io[name][s])
        slot_tiles[s] = t
        return t

    def s2_c_phase(u, nxt):
        """nn accumulation (den = row 0) + normalize into one nns tile.
        S1 steps of the next unit are interleaved to keep ACT fed and cover
        the C-phase latency before D starts."""
        k, nb_t, se, Etile, lhs_nn = u["k"], u["nb_t"], u["se"], u["et"], u["xn"]
        nns = nnsp.tile([128, 1024], FP16, tag="nns", name=f"nns_{k}")
        n_s1 = nxt["nb_t"] if nxt is not None else 0
        total_nn = sum(1 for _ in _chunks(se)) * nb_t
        s1_done = 0
        nn_done = 0
        for off, w in _chunks(se):
            nnp = ps_nn.tile([128, 512], F32, tag="nn", name=f"nn_{k}_{off}")
            for tb in range(nb_t):
                nc.tensor.matmul(nnp[:, 0:w], lhsT=lhs_nn[:, tb, :],
                                 rhs=Etile[:, tb, off:off + w],
                                 start=tb == 0, stop=tb == nb_t - 1)
                nn_done += 1
                while n_s1 and s1_done < (n_s1 * nn_done) // total_nn:
                    s1_step(nxt, s1_done)
                    s1_done += 1
            rr = rrp.tile([1, 512], F32, tag="rr")
            nc.vector.reciprocal_approx_fast(out=rr[:, 0:w], in_=nnp[0:1, 0:w])
            bc = bcp.tile([128, 512], F32, tag="bc")
            nc.gpsimd.partition_broadcast(bc[:, 0:w], rr[:, 0:w])
            nc.vector.scalar_tensor_tensor(nns[:, off:off + w], in0=nnp[:, 0:w],
                                           scalar=1.0, in1=bc[:, 0:w],
                                           op0=ALU.bypass, op1=ALU.mult)
        while s1_done < n_s1:
            s1_step(nxt, s1_done)
            s1_done += 1
        return nns

    def s1_step(u, tb):
        big = ps_big.tile([128, 1024], F32, tag="big", name=f"s1_{u['k']}_{tb}")
        se = u["se"]
        for off, w in _chunks(se):
            nc.tensor.matmul(big[:, off:off + w],
                             lhsT=u["X"][:, 128 * tb:128 * (tb + 1)],
                             rhs=u["Y"][:, off:off + w], start=True, stop=True)
        nc.scalar.activation(u["et"][:, tb, 0:se], big[:, 0:se], EXP,
                             bias=u["bias_t"][:, tb:tb + 1], scale=scale)

    def d_step(u, ub):
        se = u["se2"]
        big2 = ps_big.tile([128, 1024], F32, tag="big", name=f"d_{u['k']}_{ub}")
        for off, w in _chunks(se):
            nc.tensor.matmul(big2[:, off:off + w],
                             lhsT=u["Dlhs"][:, 128 * ub:128 * (ub + 1)],
                             rhs=u["nns"][:, off:off + w], start=True, stop=True)
        bt = btp.tile([128, 1024], FP16, tag="bt")
        nc.scalar.activation(bt[:, 0:se], big2[:, 0:se], EXP,
                             bias=u["bias_d"][:, ub:ub + 1], scale=scale)
        u["pend"].append(bt)

    def d_flush(u, ub):
        for qi, (off, w) in enumerate(_chunks(u["se2"])):
            nc.tensor.matmul(u["th"][32 * qi:32 * qi + 2, 0:w],
                             lhsT=thin2w[:, ub, :],
                             rhs=u["pend"][ub][:, off:off + w],
                             start=ub == 0, stop=ub == u["nb_u"] - 1)

    units = []
    for s in range(SLOTS):
        cb, sb = plans[s]
        se0, se1 = cb * 128, sb * 128
        t = get_slot(s)
        units.append(dict(k=2 * s, nb_t=sb, nb_u=cb, se=se0, se2=se0,
                          X=t["st"], Y=t["ct"], bias_t=t["bias_s"],
                          xn=t["xna"], Dlhs=t["ct0"], bias_d=t["bias_c"]))
        units.append(dict(k=2 * s + 1, nb_t=cb, nb_u=sb, se=se1, se2=se1,
                          X=t["ct"], Y=t["st"], bias_t=t["bias_c"],
                          xn=t["xnb"], Dlhs=t["st0"], bias_d=t["bias_s"]))

    # prologue: S1 of unit 0
    units[0]["et"] = etp.tile([128, NB, 1024], FP16, tag="et", name="et0")
    for tb in range(units[0]["nb_t"]):
        s1_step(units[0], tb)

    LAG = 2
    for j, u in enumerate(units):
        nxt = units[j + 1] if j + 1 < len(units) else None
        if nxt is not None:
            nxt["et"] = etp.tile([128, NB, 1024], FP16, tag="et",
                                 name=f"et{nxt['k']}")
        u["nns"] = s2_c_phase(u, nxt)
        u["th"] = ps_th.tile([34, 512], F32, tag="th", name=f"th_{u['k']}")
        u["pend"] = []
        for i in range(u["nb_u"]):
            d_step(u, i)
            if i >= LAG:
                d_flush(u, i - LAG)
        for ub in range(max(0, u["nb_u"] - LAG), u["nb_u"]):
            d_flush(u, ub)
        for qi, (off, w) in enumerate(_chunks(u["se2"])):
            nc.vector.tensor_copy(th_sb[32 * qi:32 * qi + 2, u["k"], 0:w],
                                  u["th"][32 * qi:32 * qi + 2, 0:w])

    # ---- final: batched loss over [units, 2, 512] ----
    den8 = fin.tile([NUNITS, 2, 512], F32, tag="den8")
    num8 = fin.tile([NUNITS, 2, 512], F32, tag="num8")
    nc.sync.dma_start(out=den8[:, 0, :], in_=th_sb[0:1, :, :])
    nc.sync.dma_start(out=den8[:, 1, :], in_=th_sb[32:33, :, :])
    nc.sync.dma_start(out=num8[:, 0, :], in_=th_sb[1:2, :, :])
    nc.sync.dma_start(out=num8[:, 1, :], in_=th_sb[33:34, :, :])
    rden = fin.tile([NUNITS, 2, 512], F32, tag="rden")
    scr = fin.tile([NUNITS, 2, 512], F32, tag="scr")
    nc.vector.reciprocal_approx_accurate(out=rden, in_=den8, scratch=scr)
    idx = fin.tile([NUNITS, 2, 512], F32, tag="idx")
    nc.vector.tensor_mul(idx, num8, rden)
    ierr = fin.tile([NUNITS, 2, 512], F32, tag="ierr")
    nc.vector.tensor_sub(ierr, idx, iota_t)
    tmp = fin.tile([NUNITS, 2, 512], F32, tag="tmp")
    nc.vector.tensor_mul(tmp, ierr, masks_t)
    sq = fin.tile([NUNITS, 2, 512], F32, tag="sq")
    sums = fin.tile([NUNITS, 1], F32, tag="sums")
    nc.vector.scalar_tensor_tensor(sq, in0=tmp, scalar=1.0, in1=ierr,
                                   op0=ALU.bypass, op1=ALU.mult, accum_out=sums)
    loss = fin.tile([NUNITS, 1], F32, tag="loss")
    nc.vector.tensor_mul(loss, sums, rlens_t)
    nc.sync.dma_start(out=io["loss8"], in_=loss)


def _build_program(plans):
    key = tuple(plans)
    if key in _PROGRAM_CACHE:
        return _PROGRAM_CACHE[key]
    nc = bacc.Bacc("TRN2", target_bir_lowering=False, debug=False,
                   num_devices=NCORES)
    io = {
        "ct": nc.dram_tensor("ct", [SLOTS, D, M], FP16, kind="ExternalInput").ap(),
        "st": nc.dram_tensor("st", [SLOTS, D, N], FP16, kind="ExternalInput").ap(),
        "ct0": nc.dram_tensor("ct0", [SLOTS, D, M], FP16, kind="ExternalInput").ap(),
        "st0": nc.dram_tensor("st0", [SLOTS, D, N], FP16, kind="ExternalInput").ap(),
        "xna": nc.dram_tensor("xna", [SLOTS, 128, NB, 128], FP16, kind="ExternalInput").ap(),
        "xnb": nc.dram_tensor("xnb", [SLOTS, 128, NB, 128], FP16, kind="ExternalInput").ap(),
        "bias_c": nc.dram_tensor("bias_c", [SLOTS, 128, NB], F32, kind="ExternalInput").ap(),
        "bias_s": nc.dram_tensor("bias_s", [SLOTS, 128, NB], F32, kind="ExternalInput").ap(),
        "thin2w": nc.dram_tensor("thin2w", [128, NB, 2], FP16, kind="ExternalInput").ap(),
        "iota8": nc.dram_tensor("iota8", [NUNITS, M], F32, kind="ExternalInput").ap(),
        "masks8": nc.dram_tensor("masks8", [NUNITS, M], F32, kind="ExternalInput").ap(),
        "rlens": nc.dram_tensor("rlens", [NUNITS, 1], F32, kind="ExternalInput").ap(),
        "loss8": nc.dram_tensor("loss8", [NUNITS, 1], F32, kind="ExternalOutput").ap(),
    }
    from contextlib import ExitStack
    with tile.TileContext(nc) as tc:
        with ExitStack() as ctx:
            _emit(nc, tc, ctx, io, plans)
    nc.compile()
    _PROGRAM_CACHE[key] = nc
    return nc


def _host_prep(clip_emb, clip_mask, clip_lens, sent_emb, sent_mask, sent_lens):
    """Sorted batch->(core,slot) assignment, per-slot plans, per-core inputs."""
    cb_all = np.ceil(clip_lens / 128).astype(int)
    sb_all = np.ceil(sent_lens / 128).astype(int)
    order = np.argsort(-(cb_all + sb_all) * 1000 - cb_all)  # big batches first
    plans = []
    assign = {}
    for s in range(SLOTS):
        grp = order[8 * s:8 * s + 8]
        plans.append((int(cb_all[grp].max()), int(sb_all[grp].max())))
        for core, b in enumerate(grp):
            assign[(core, s)] = int(b)

    sq_c = np.einsum("bmd,bmd->bm", clip_emb, clip_emb)
    sq_s = np.einsum("bnd,bnd->bn", sent_emb, sent_emb)
    bias_c = (-sq_c / D + PEN * (1.0 - clip_mask)).astype(np.float32)
    bias_s = (-sq_s / D + PEN * (1.0 - sent_mask)).astype(np.float32)
    thin2w = np.zeros((128, NB, 2), np.float16)
    thin2w[:, :, 0] = 1.0
    thin2w[:, :, 1] = (np.arange(128)[:, None] + 128 * np.arange(NB)[None, :])
    iota8 = np.broadcast_to(np.arange(M, dtype=np.float32), (NUNITS, M)).copy()

    in_maps = []
    for core in range(NCORES):
        bs = [assign[(core, s)] for s in range(SLOTS)]
        ce = clip_emb[bs]
        se = sent_emb[bs]
        ct = np.ascontiguousarray(ce.transpose(0, 2, 1)).astype(np.float16)
        st = np.ascontiguousarray(se.transpose(0, 2, 1)).astype(np.float16)
        ct0 = np.zeros_like(ct)
        ct0[:, 1:, :] = ct[:, :127, :]
        st0 = np.zeros_like(st)
        st0[:, 1:, :] = st[:, :127, :]
        xt = se.reshape(SLOTS, NB, 128, D).transpose(0, 2, 1, 3)
        xna = np.zeros((SLOTS, 128, NB, D), np.float16)
        xna[..., 1:] = xt[..., :127]
        xna[..., 0] = 1.0
        xbt = ce.reshape(SLOTS, NB, 128, D).transpose(0, 2, 1, 3)
        xnb = np.zeros((SLOTS, 128, NB, D), np.float16)
        xnb[..., 1:] = xbt[..., :127]
        xnb[..., 0] = 1.0

        masks8 = np.empty((NUNITS, M), np.float32)
        rlens = np.empty((NUNITS, 1), np.float32)
        for s, b in enumerate(bs):
            masks8[2 * s + 0] = clip_mask[b]
            masks8[2 * s + 1] = sent_mask[b]
            rlens[2 * s + 0] = 1.0 / clip_lens[b]
            rlens[2 * s + 1] = 1.0 / sent_lens[b]
        in_maps.append({
            "ct": ct, "st": st, "ct0": ct0, "st0": st0,
            "xna": xna, "xnb": xnb,
            "bias_c": np.ascontiguousarray(
                bias_c[bs].reshape(SLOTS, NB, 128).transpose(0, 2, 1)),
            "bias_s": np.ascontiguousarray(
                bias_s[bs].reshape(SLOTS, NB, 128).transpose(0, 2, 1)),
            "thin2w": thin2w,
            "iota8": iota8,
            "masks8": masks8,
            "rlens": rlens,
        })
    return in_maps, assign, plans


def kernel(clip_emb, clip_mask, clip_lens, sent_emb, sent_mask, sent_lens):
    global LAST_RESULT
    clip_emb = np.asarray(clip_emb, np.float32)
    sent_emb = np.asarray(sent_emb, np.float32)
    clip_mask = np.asarray(clip_mask, np.float32)
    sent_mask = np.asarray(sent_mask, np.float32)
    clip_lens = np.asarray(clip_lens, np.float32)
    sent_lens = np.asarray(sent_lens, np.float32)

    in_maps, _, plans = _host_prep(clip_emb, clip_mask, clip_lens,
                                   sent_emb, sent_mask, sent_lens)
    nc = _build_program(plans)
    res = run_bass_kernel_spmd(nc, in_maps, list(range(NCORES)))
    LAST_RESULT = res

    rows = np.stack([res.results[c]["loss8"].reshape(NUNITS) for c in range(NCORES)])
    clip_loss = rows[:, 0::2].mean()
    sent_loss = rows[:, 1::2].mean()
    return (np.float32(clip_loss), np.float32(sent_loss))
